# revision 18
# baseline (speedup 1.0000x reference)
"""Combined CE + Dice loss on 8 TRN2 NeuronCores (Bass/Tile, SPMD data-parallel).

Reference computation (N=16, C=4, H=W=512):
  loss_ce   = -mean(log_softmax(preds, axis=1) gathered at targets)
  inter_i   = sum(preds[i] == targets[i])          (broadcast [C,H,W] vs [H,W])
  union     = preds.sum() + targets.sum()
  loss_dice = 1 - mean((2*inter + S) / (union + S))
  out       = 0.5*loss_ce + 0.5*loss_dice

Sharding: batch dim N=16 -> 2 samples per core.  Each core streams its
8 MiB of preds once and produces tiny partial accumulators:
  sum(lse), sum(x_t), per-sample sum(preds==t), sum(preds), sum(t)
which the host combines into the final scalar (the "all-reduce").

On-device layout per sample (DMA-minimal: no on-chip replication of t):
  x [128, 4*2048] f32  - partition p holds pixels [2048p, 2048(p+1)) of all
                         four class planes as four 2048-wide segments
  t [128, 2048]  fp16  - same pixel->partition map (0..3 exact in fp16)
Per sample:
  ACT exp:  e = exp(x) -> fp16, one op
  DVE adds: s = (e0+e1)+(e2+e3) per pixel, fp16 2x mode
  ACT copy: scx = fp16(x) with accum_out -> sum(preds); scx feeds q
  ACT ln:   ln(s) with accum_out -> sum(lse)
  DVE q:    per class c: (t == c) * scx_seg_c with accum_out -> sum(x_t)
  DVE i:    per class c: (x_seg_c * 1) == t with accum_out -> inter (fp32 exact)
  DVE tsum: (t * 1) with accum_out -> sum(t)
"""

import numpy as np
from contextlib import ExitStack

import ml_dtypes

import concourse.bass as bass
import concourse.tile as tile
from concourse import bacc, mybir
from concourse.bass_utils import run_bass_kernel_spmd

# Problem shape (hardcoded per contract; kernel.py must be self-contained).
N, C, H, W = 16, 4, 512, 512
NCORES = 8
NLOC = N // NCORES          # samples per core
PIX = H * W                 # pixels per sample
SEG = PIX // 128            # 2048 pixels per partition per sample

ALPHA = 0.5
SMOOTH = 1e-08

F32 = mybir.dt.float32
F16 = mybir.dt.float16
AF = mybir.ActivationFunctionType
ALU = mybir.AluOpType

_CACHE = {}


def _build_nc():
    nc = bacc.Bacc(
        "TRN2", target_bir_lowering=False, debug=False, num_devices=NCORES
    )

    preds_d = nc.dram_tensor("preds", [NLOC, C, 128, SEG], F32, kind="ExternalInput")
    tgt_d = nc.dram_tensor("tgt", [NLOC, 128, SEG], F16, kind="ExternalInput")

    acc_lse_d = nc.dram_tensor("acc_lse", [128, NLOC], F32, kind="ExternalOutput")
    acc_q_d = nc.dram_tensor("acc_q", [128, NLOC * C], F32, kind="ExternalOutput")
    acc_i_d = nc.dram_tensor("acc_i", [128, NLOC * C], F32, kind="ExternalOutput")
    acc_x_d = nc.dram_tensor("acc_x", [128, NLOC], F32, kind="ExternalOutput")
    acc_t_d = nc.dram_tensor("acc_t", [128, NLOC], F32, kind="ExternalOutput")

    with tile.TileContext(nc) as tc, ExitStack() as ctx:
        acc_pool = ctx.enter_context(tc.tile_pool(name="acc", bufs=1))
        x_pool = ctx.enter_context(tc.tile_pool(name="x", bufs=2))
        t_pool = ctx.enter_context(tc.tile_pool(name="t", bufs=2))
        e_pool = ctx.enter_context(tc.tile_pool(name="e", bufs=2))
        cx_pool = ctx.enter_context(tc.tile_pool(name="cx", bufs=2))
        s_pool = ctx.enter_context(tc.tile_pool(name="s", bufs=2))
        scr_pool = ctx.enter_context(tc.tile_pool(name="scr", bufs=3))

        acc_lse_t = acc_pool.tile([128, NLOC], F32)
        acc_q_t = acc_pool.tile([128, NLOC * C], F32)
        acc_i_t = acc_pool.tile([128, NLOC * C], F32)
        acc_x_t = acc_pool.tile([128, NLOC], F32)
        acc_t_t = acc_pool.tile([128, NLOC], F32)

        def seg(tile_, c):
            return tile_[:, SEG * c : SEG * (c + 1)]

        for i in range(NLOC):
            xb = x_pool.tile([128, C * SEG], F32)
            for c in range(C):
                nc.sync.dma_start(seg(xb, c), preds_d.ap()[i, c])
            tb = t_pool.tile([128, SEG], F16)
            nc.sync.dma_start(tb[:], tgt_d.ap()[i])

            # ACT, per-sample order exp -> copy -> ln limits table swaps
            # (copy lives in every table set).
            eb = e_pool.tile([128, C * SEG], F16)
            nc.scalar.activation(eb[:], xb[:], AF.Exp)

            scx = cx_pool.tile([128, C * SEG], F16)
            nc.scalar.activation(
                scx[:], xb[:], AF.Copy, accum_out=acc_x_t[:, i : i + 1]
            )

            s1 = s_pool.tile([128, SEG], F16, tag="stmp")
            nc.vector.tensor_add(s1[:], seg(eb, 0), seg(eb, 1))
            s2 = s_pool.tile([128, SEG], F16, tag="stmp")
            nc.vector.tensor_add(s2[:], seg(eb, 2), seg(eb, 3))
            sb = s_pool.tile([128, SEG], F16, tag="s")
            nc.vector.tensor_add(sb[:], s1[:], s2[:])

            lsb = scr_pool.tile([128, SEG], F16, tag="ls")
            nc.scalar.activation(
                lsb[:], sb[:], AF.Ln, accum_out=acc_lse_t[:, i : i + 1]
            )

            # sum(t) on DVE (fp16 single-src -> fast mode)
            st = scr_pool.tile([128, SEG], F16, tag="st")
            nc.vector.tensor_scalar(
                st[:], tb[:], 1.0, None, ALU.mult, ALU.add,
                accum_out=acc_t_t[:, i : i + 1],
            )

            for c in range(C):
                col = i * C + c
                # sum(x_t): (t == c) * x  (all-fp16 operands)
                scq = scr_pool.tile([128, SEG], F16, tag="scq")
                nc.vector.scalar_tensor_tensor(
                    scq[:],
                    tb[:],
                    float(c),
                    seg(scx, c),
                    ALU.is_equal,
                    ALU.mult,
                    accum_out=acc_q_t[:, col : col + 1],
                )
                # dice intersection: (x * 1) == t with fp32 x (exact compare)
                sci = scr_pool.tile([128, SEG], F16, tag="sci")
                nc.vector.scalar_tensor_tensor(
                    sci[:],
                    seg(xb, c),
                    1.0,
                    tb[:],
                    ALU.mult,
                    ALU.is_equal,
                    accum_out=acc_i_t[:, col : col + 1],
                )

        nc.sync.dma_start(acc_lse_d.ap(), acc_lse_t[:])
        nc.sync.dma_start(acc_q_d.ap(), acc_q_t[:])
        nc.sync.dma_start(acc_i_d.ap(), acc_i_t[:])
        nc.sync.dma_start(acc_x_d.ap(), acc_x_t[:])
        nc.sync.dma_start(acc_t_d.ap(), acc_t_t[:])

    nc.compile()
    return nc


def kernel(preds: np.ndarray, targets: np.ndarray) -> np.ndarray:
    assert preds.shape == (N, C, H, W) and targets.shape == (N, H, W)
    if "nc" not in _CACHE:
        _CACHE["nc"] = _build_nc()
    nc = _CACHE["nc"]

    preds = np.ascontiguousarray(preds, dtype=np.float32)
    tgt_f = np.ascontiguousarray(targets.astype(np.float16))

    preds_r = preds.reshape(NCORES, NLOC, C, 128, SEG)
    tgt_r = tgt_f.reshape(NCORES, NLOC, 128, SEG)

    in_maps = [{"preds": preds_r[k], "tgt": tgt_r[k]} for k in range(NCORES)]
    res = run_bass_kernel_spmd(nc, in_maps, list(range(NCORES))).results

    lse_sum = 0.0
    q_sum = 0.0
    x_sum = 0.0
    t_sum = 0.0
    inter = np.zeros(N, dtype=np.float64)
    for k in range(NCORES):
        r = res[k]
        lse_sum += r["acc_lse"].astype(np.float64).sum()
        q_sum += r["acc_q"].astype(np.float64).sum()
        x_sum += r["acc_x"].astype(np.float64).sum()
        t_sum += r["acc_t"].astype(np.float64).sum()
        acc_i = r["acc_i"].astype(np.float64)
        for i in range(NLOC):
            inter[k * NLOC + i] = acc_i[:, i * C : (i + 1) * C].sum()

    n_pix = float(N * H * W)
    loss_ce = (lse_sum - q_sum) / n_pix
    union = x_sum + t_sum
    dice = (2.0 * inter + SMOOTH) / (union + SMOOTH)
    loss_dice = 1.0 - dice.mean()
    out = ALPHA * loss_ce + (1.0 - ALPHA) * loss_dice
    return np.float32(out)
